# revision 1
# baseline (speedup 1.0000x reference)
"""Trainium2 SPMD kernel for H2OLlama attention (GQA + RoPE + causal softmax + o_proj).

Sharding (8 cores): core = b*4 + g  (b in {0,1} batch, g in {0..3} head group).
Each core handles one batch element, 8 q-heads and its 2 kv-heads; host sums the
4 o_proj partials per batch and transposes back.

v2 redesign vs baseline:
- V projection computed transposed (K-style, N=512 matmuls) + PE 128x128
  transposes, instead of 512 cold N=256 matmuls.
- Softmax denominator: P blocks accumulated elementwise on DVE (Ptot), then one
  GpSimd partition_all_reduce (fused reduce+broadcast) + DVE fast reciprocal —
  removes 320 row-sum matmuls, 32 broadcast matmuls and the 3.1us [1,512]
  reciprocals that stalled the PE.
- Phase overlap: attention waves qt=0,1 are emitted inside the projection scope
  (they only need half-0 projections); o_proj emitted last, fills PE gaps.
- RoPE tmp t2 written in-place into the rotation PSUM tile; t1 into the proj
  PSUM tile (saves SBUF + DVE traffic).
"""

import math
import sys

import numpy as np

sys.path.insert(0, "/opt/trn_rl_repo")

import ml_dtypes

import concourse.bass as bass
import concourse.tile as tile
from concourse import bacc, bass_isa, mybir
from concourse.bass_utils import run_bass_kernel_spmd

BF16 = mybir.dt.bfloat16
F32 = mybir.dt.float32

HIDDEN = 4096
N_HEADS = 32
N_KV_HEADS = 8
HEAD_DIM = 128
B, S = 2, 2048
ROPE_THETA = 10000.0

N_CORES = 8
HEADS_PER_CORE = N_HEADS // 4  # 8 q heads per core (4 head groups)
KV_PER_CORE = N_KV_HEADS // 4  # 2 kv heads per core
QDIM = HEADS_PER_CORE * HEAD_DIM  # 1024
KVDIM = KV_PER_CORE * HEAD_DIM  # 256
HC = HIDDEN // 128  # 32 hidden chunks
TT128 = S // 128  # 16 token tiles of 128
SCALE = 1.0 / math.sqrt(HEAD_DIM)

_BUILD_CACHE = {}


def _build_program():
    nc = bacc.Bacc("TRN2", target_bir_lowering=False, debug=False, num_devices=N_CORES)

    xt_d = nc.dram_tensor("xt", [128, HC, S], BF16, kind="ExternalInput")
    wq_d = nc.dram_tensor("wq", [HEADS_PER_CORE, 128, HC, 128], BF16, kind="ExternalInput")
    wk_d = nc.dram_tensor("wk", [KV_PER_CORE, 128, HC, 128], BF16, kind="ExternalInput")
    wv_d = nc.dram_tensor("wv", [KV_PER_CORE, 128, HC, 128], BF16, kind="ExternalInput")
    wo_d = nc.dram_tensor("wo", [128, HC, HEADS_PER_CORE, 128], BF16, kind="ExternalInput")
    cos_d = nc.dram_tensor("cosT", [128, S], F32, kind="ExternalInput")
    sin_d = nc.dram_tensor("sinT", [128, S], F32, kind="ExternalInput")
    rt_d = nc.dram_tensor("rt", [128, 128], BF16, kind="ExternalInput")
    idn_d = nc.dram_tensor("idn", [128, 128], BF16, kind="ExternalInput")
    mask_d = nc.dram_tensor("maskb", [128, 1024], BF16, kind="ExternalInput")
    out_d = nc.dram_tensor("out", [HIDDEN, S], BF16, kind="ExternalOutput")

    with tile.TileContext(nc) as tc:
        _kernel_body(nc, tc, xt_d, wq_d, wk_d, wv_d, wo_d, cos_d, sin_d, rt_d, idn_d, mask_d, out_d)

    nc.compile()
    return nc


def _kernel_body(nc, tc, xt_d, wq_d, wk_d, wv_d, wo_d, cos_d, sin_d, rt_d, idn_d, mask_d, out_d):
    EXP = mybir.ActivationFunctionType.Exp

    with (
        tc.tile_pool(name="qr", bufs=1) as qr_pool,
        tc.tile_pool(name="kr", bufs=1) as kr_pool,
        tc.tile_pool(name="vv", bufs=1) as v_pool,
        tc.tile_pool(name="aa", bufs=1) as a_pool,
        tc.tile_pool(name="consts", bufs=1) as const_pool,
        # attention working pools (span wave A in proj scope and wave B after)
        tc.tile_pool(name="pp", bufs=5) as p_pool,
        tc.tile_pool(name="ptot", bufs=4) as ptot_pool,
        tc.tile_pool(name="rr", bufs=2) as r_pool,
        tc.tile_pool(name="sps", bufs=3, space="PSUM") as s_psum,
        tc.tile_pool(name="ops", bufs=2, space="PSUM") as o_psum,
    ):
        qr = [qr_pool.tile([128, S], BF16, tag=f"qr{h}", name=f"qr{h}") for h in range(HEADS_PER_CORE)]
        kr = [kr_pool.tile([128, S], BF16, tag=f"kr{k}", name=f"kr{k}") for k in range(KV_PER_CORE)]
        vt = v_pool.tile([128, TT128, KVDIM], BF16, tag="v", name="vt")
        at = [a_pool.tile([128, S], BF16, tag=f"a{h}", name=f"a{h}") for h in range(HEADS_PER_CORE)]

        rt_sb = const_pool.tile([128, 128], BF16, tag="rt", name="rt_sb")
        idn_sb = const_pool.tile([128, 128], BF16, tag="idn", name="idn_sb")
        mask_sb = const_pool.tile([128, 1024], BF16, tag="mask", name="mask_sb")
        ones_sb = const_pool.tile([128, 128], BF16, tag="ones", name="ones_sb")
        nc.vector.memset(ones_sb[:], 1.0)

        # ---------------- attention block emitter ----------------
        def attn(h, qt):
            kvl = h // 4
            nj = 4 * qt + 4
            q_rhs = qr[h][:, qt * 512 : (qt + 1) * 512]
            o_ps = o_psum.tile([128, 512], F32, tag="o", name="o_ps")
            ptot = ptot_pool.tile([128, 512], BF16, tag="ptot", name="ptot")
            pts = {}

            def emit_score(j):
                s_ps = s_psum.tile([128, 512], F32, tag="s", name="s_ps")
                nc.tensor.matmul(
                    s_ps[:],
                    lhsT=kr[kvl][:, j * 128 : (j + 1) * 128],
                    rhs=q_rhs,
                    start=True,
                    stop=True,
                )
                p_t = p_pool.tile([128, 512], BF16, tag="p", name="p_t")
                nc.scalar.activation(p_t[:], s_ps[:], EXP, scale=SCALE)
                if j >= 4 * qt:  # diagonal block: multiplicative causal mask
                    a = j - 4 * qt
                    nc.vector.tensor_mul(
                        p_t[:], p_t[:], mask_sb[:, 512 - a * 128 : 1024 - a * 128]
                    )
                pts[j] = p_t

            emit_score(0)
            emit_score(1)
            if nj > 2:
                emit_score(2)
            for j in range(nj):
                if j + 3 < nj:
                    emit_score(j + 3)
                p_t = pts.pop(j)
                nc.tensor.matmul(
                    o_ps[:],
                    lhsT=vt[:, j, kvl * 128 : (kvl + 1) * 128],
                    rhs=p_t[:],
                    start=(j == 0),
                    stop=(j == nj - 1),
                )
                if j == 0:
                    nc.vector.tensor_copy(ptot[:], p_t[:])
                else:
                    nc.vector.tensor_add(ptot[:], ptot[:], p_t[:])
            # copy unnormalized output to SBUF immediately so the PSUM bank
            # recycles without waiting for the normalization chain
            o_sb = r_pool.tile([128, 512], F32, tag="osb", name="o_sb")
            nc.vector.tensor_copy(o_sb[:], o_ps[:])
            # fused row-sum + broadcast: ones[128,128].T @ ptot -> every row = r
            b_ps = s_psum.tile([128, 512], F32, tag="s", name="b_ps")
            nc.tensor.matmul(b_ps[:], lhsT=ones_sb[:], rhs=ptot[:], start=True, stop=True)
            ri = r_pool.tile([128, 512], F32, tag="ri", name="ri")
            nc.vector.reciprocal_approx_fast(ri[:], b_ps[:])
            nc.vector.tensor_mul(at[h][:, qt * 512 : (qt + 1) * 512], o_sb[:], ri[:])

        # ================= projections (+ attention wave A) =================
        with (
            tc.tile_pool(name="xt", bufs=1) as x_pool,
            tc.tile_pool(name="wqk", bufs=2) as wqk_pool,
            tc.tile_pool(name="cs", bufs=1) as cs_pool,
            tc.tile_pool(name="raw", bufs=2) as raw_pool,
            tc.tile_pool(name="t1p", bufs=2) as t1_pool,
            tc.tile_pool(name="qkps", bufs=1, space="PSUM") as qk_psum,
            tc.tile_pool(name="rotps", bufs=1, space="PSUM") as rot_psum,
        ):
            for half in range(2):
                toff = half * 1024
                # prefetch the first two douts' weights ahead of cos/sin/consts
                # so the very first matmul starts as early as possible
                douts = (
                    [("k", i) for i in range(KV_PER_CORE)]
                    + [("v", i) for i in range(KV_PER_CORE)]
                    + [("q", i) for i in range(HEADS_PER_CORE)]
                )
                wsrc = {"k": wk_d, "v": wv_d, "q": wq_d}

                def w_dma(w_sb, kind, idx):
                    # split in two so the first LDWEIGHTS waits on half the bytes
                    src = wsrc[kind].ap()[idx]
                    nc.scalar.dma_start(w_sb[:, 0:16, :], src[:, 0:16, :])
                    nc.scalar.dma_start(w_sb[:, 16:HC, :], src[:, 16:HC, :])

                pre_w = []
                xs = []
                # interleave the critical-path DMAs: first w half, first x chunk
                w0 = wqk_pool.tile([128, HC, 128], BF16, tag="wqk", name="w_sb")
                x0 = x_pool.tile([128, 4, 1024], BF16, tag="x0", name="x0")
                nc.scalar.dma_start(w0[:, 0:16, :], wsrc[douts[0][0]].ap()[douts[0][1]][:, 0:16, :])
                nc.sync.dma_start(x0[:], xt_d.ap()[:, 0:4, toff : toff + 1024])
                nc.scalar.dma_start(w0[:, 16:HC, :], wsrc[douts[0][0]].ap()[douts[0][1]][:, 16:HC, :])
                pre_w.append(w0)
                xs.append(x0)
                w1 = wqk_pool.tile([128, HC, 128], BF16, tag="wqk", name="w_sb")
                w_dma(w1, *douts[1])
                pre_w.append(w1)

                cos_sb = cs_pool.tile([128, 1024], F32, tag="cos", name="cos_sb")
                sin_sb = cs_pool.tile([128, 1024], F32, tag="sin", name="sin_sb")
                nc.scalar.dma_start(cos_sb[:], cos_d.ap()[:, toff : toff + 1024])
                nc.scalar.dma_start(sin_sb[:], sin_d.ap()[:, toff : toff + 1024])
                if half == 0:
                    nc.scalar.dma_start(rt_sb[:], rt_d.ap())
                    nc.scalar.dma_start(idn_sb[:], idn_d.ap())
                    nc.scalar.dma_start(mask_sb[:], mask_d.ap())

                for cg in range(1, 8):
                    xtile = x_pool.tile([128, 4, 1024], BF16, tag=f"x{cg}", name=f"x{cg}")
                    nc.sync.dma_start(
                        xtile[:], xt_d.ap()[:, cg * 4 : (cg + 1) * 4, toff : toff + 1024]
                    )
                    xs.append(xtile)

                def xsl(hc, lo, sz):
                    return xs[hc // 4][:, hc % 4, lo : lo + sz]

                wave_a = [(h, 0) for h in range(HEADS_PER_CORE)] + [
                    (h, 1) for h in range(HEADS_PER_CORE)
                ]
                # front-load wave-A groups: early half-1 douts race the x DMA,
                # so they get more attention filler
                wa_take = [3, 3, 3, 3, 1, 1, 1, 1, 0, 0, 0, 0]
                wa_off = [sum(wa_take[:i]) for i in range(len(wa_take))]
                for di, (kind, idx) in enumerate(douts):
                    if half == 1:
                        for hq in wave_a[wa_off[di] : wa_off[di] + wa_take[di]]:
                            attn(*hq)
                    if di < 2:
                        w_sb = pre_w[di]
                    else:
                        w_sb = wqk_pool.tile([128, HC, 128], BF16, tag="wqk", name="w_sb")
                        w_dma(w_sb, kind, idx)

                    pss = [
                        qk_psum.tile([128, 512], F32, tag=f"qk{tt}", name=f"qkps{tt}")
                        for tt in range(2)
                    ]
                    for hc in range(HC):
                        for tt in range(2):
                            nc.tensor.matmul(
                                pss[tt][:],
                                lhsT=w_sb[:, hc, :],
                                rhs=xsl(hc, tt * 512, 512),
                                start=(hc == 0),
                                stop=(hc == HC - 1),
                            )
                    if kind == "v":
                        # vT chunk [d=128, tok 512] -> transpose to vt [tok, d]
                        for tt in range(2):
                            vraw = raw_pool.tile([128, 512], BF16, tag="raw", name="vraw")
                            nc.vector.tensor_copy(vraw[:], pss[tt][:])
                            for c in range(4):
                                jb = half * 8 + tt * 4 + c
                                tp = s_psum.tile([128, 128], BF16, tag="s", name="tp")
                                nc.tensor.transpose(
                                    tp[:], vraw[:, c * 128 : (c + 1) * 128], idn_sb[:]
                                )
                                nc.vector.tensor_copy(
                                    vt[:, jb, idx * 128 : (idx + 1) * 128], tp[:]
                                )
                    else:
                        dst = qr[idx] if kind == "q" else kr[idx]
                        for tt in range(2):
                            g0 = tt * 512
                            raw = raw_pool.tile([128, 512], BF16, tag="raw", name="raw")
                            nc.vector.tensor_copy(raw[:], pss[tt][:])
                            rot = rot_psum.tile([128, 512], F32, tag="rot", name="rot")
                            nc.tensor.matmul(
                                rot[:], lhsT=rt_sb[:], rhs=raw[:], start=True, stop=True
                            )
                            # t2 = rot * sin, in place in PSUM
                            nc.vector.tensor_mul(rot[:], rot[:], sin_sb[:, g0 : g0 + 512])
                            t1 = t1_pool.tile([128, 512], F32, tag="t1", name="t1")
                            nc.vector.tensor_mul(t1[:], raw[:], cos_sb[:, g0 : g0 + 512])
                            nc.vector.tensor_add(
                                dst[:, toff + g0 : toff + g0 + 512], t1[:], rot[:]
                            )


        # ================= phase 2: wave B + o_proj =================
        with (
            tc.tile_pool(name="wo", bufs=1) as wo_pool,
            tc.tile_pool(name="oout", bufs=4) as out_pool,
            tc.tile_pool(name="outps", bufs=3, space="PSUM") as out_psum,
        ):
            # on the sync queue: a scalar-queue trigger here would block wave-B
            # exp instructions behind its slot-wait in the strict ACT FIFO
            wo_sb = wo_pool.tile([128, HC, HEADS_PER_CORE, 128], BF16, tag="wo", name="wo_sb")
            for c in range(4):
                nc.sync.dma_start(
                    wo_sb[:, c * 8 : (c + 1) * 8, :, :],
                    wo_d.ap()[:, c * 8 : (c + 1) * 8, :, :],
                )

            def oproj_unit(tt, do):
                ps = out_psum.tile([128, 512], F32, tag="op", name="op")
                for a in range(HEADS_PER_CORE):
                    nc.tensor.matmul(
                        ps[:],
                        lhsT=wo_sb[:, do, a, :],
                        rhs=at[a][:, tt * 512 : (tt + 1) * 512],
                        start=(a == 0),
                        stop=(a == HEADS_PER_CORE - 1),
                    )
                ot = out_pool.tile([128, 512], BF16, tag="ot", name="ot")
                nc.vector.tensor_copy(ot[:], ps[:])
                nc.sync.dma_start(
                    out_d.ap()[do * 128 : (do + 1) * 128, tt * 512 : (tt + 1) * 512],
                    ot[:],
                )

            # interleave wave-B attention with o_proj units so the PE always
            # has filler work while exp/reduce chains run on ACT/GpSimd/DVE
            wave_b = [(h, 2) for h in range(HEADS_PER_CORE)] + [
                (h, 3) for h in range(HEADS_PER_CORE)
            ]
            units = [(tt, do) for tt in range(3) for do in range(HC)]  # tt=0,1,2
            ui = 0
            for gi, hq in enumerate(wave_b):
                attn(*hq)
                take = 5 if gi < 8 else 7
                for _ in range(take):
                    if ui < len(units):
                        oproj_unit(*units[ui])
                        ui += 1
            while ui < len(units):
                oproj_unit(*units[ui])
                ui += 1
            for do in range(HC):
                oproj_unit(3, do)


# ======================= host-side sharding =======================


def _rope_tables(position_ids_b):
    pos = position_ids_b.astype(np.float32)  # [S]
    inv_freq = 1.0 / (ROPE_THETA ** (np.arange(0, HEAD_DIM, 2, dtype=np.float32) / HEAD_DIM))
    freqs = pos[:, None] * inv_freq[None, :]  # [S, 64]
    emb = np.concatenate([freqs, freqs], axis=1)  # [S, 128]
    cosT = np.ascontiguousarray(np.cos(emb).T.astype(np.float32))  # [128, S]
    sinT = np.ascontiguousarray(np.sin(emb).T.astype(np.float32))
    return cosT, sinT


def _shared_consts():
    rt = np.zeros((128, 128), dtype=ml_dtypes.bfloat16)
    idx = np.arange(64)
    rt[idx, idx + 64] = 1.0  # RT[j, j+64] = +1  (j < 64)
    rt[idx + 64, idx] = -1.0  # RT[j+64, j] = -1
    idn = np.eye(128, dtype=ml_dtypes.bfloat16)
    maskb = np.zeros((128, 1024), dtype=ml_dtypes.bfloat16)
    k = np.arange(128)[:, None]
    c = np.arange(1024)[None, :]
    maskb[:] = (c >= k + 512).astype(ml_dtypes.bfloat16)
    return rt, idn, maskb


def kernel(hidden_states, position_ids, Wq, Wk, Wv, Wo):
    bf16 = ml_dtypes.bfloat16
    if "nc" not in _BUILD_CACHE:
        _BUILD_CACHE["nc"] = _build_program()
    nc = _BUILD_CACHE["nc"]

    rt, idn, maskb = _shared_consts()
    Wq16, Wk16, Wv16, Wo16 = (w.astype(bf16) for w in (Wq, Wk, Wv, Wo))

    xts, coss, sins = [], [], []
    for b in range(B):
        xb = np.asarray(hidden_states[b], dtype=np.float32).T.astype(bf16)  # [4096, S]
        xt = np.ascontiguousarray(xb.reshape(HC, 128, S).transpose(1, 0, 2))  # [128, 32, S]
        xts.append(xt)
        cosT, sinT = _rope_tables(np.asarray(position_ids[b]))
        coss.append(cosT)
        sins.append(sinT)

    in_maps = []
    for core in range(N_CORES):
        b, g = core // 4, core % 4
        wq = np.ascontiguousarray(
            Wq16[:, g * QDIM : (g + 1) * QDIM].reshape(HC, 128, HEADS_PER_CORE, 128).transpose(2, 1, 0, 3)
        )
        wk = np.ascontiguousarray(
            Wk16[:, g * KVDIM : (g + 1) * KVDIM].reshape(HC, 128, KV_PER_CORE, 128).transpose(2, 1, 0, 3)
        )
        wv = np.ascontiguousarray(
            Wv16[:, g * KVDIM : (g + 1) * KVDIM].reshape(HC, 128, KV_PER_CORE, 128).transpose(2, 1, 0, 3)
        )
        wo = np.ascontiguousarray(
            Wo16[g * QDIM : (g + 1) * QDIM, :].reshape(HEADS_PER_CORE, 128, HC, 128).transpose(1, 2, 0, 3)
        )
        in_maps.append(
            {
                "xt": xts[b],
                "wq": wq,
                "wk": wk,
                "wv": wv,
                "wo": wo,
                "cosT": coss[b],
                "sinT": sins[b],
                "rt": rt,
                "idn": idn,
                "maskb": maskb,
            }
        )

    res = run_bass_kernel_spmd(nc, in_maps, list(range(N_CORES))).results

    out = np.empty((B, S, HIDDEN), dtype=np.float32)
    for b in range(B):
        acc = res[4 * b]["out"].astype(np.float32)
        for g in range(1, 4):
            acc = acc + res[4 * b + g]["out"]
        out[b] = acc.T
    return out



# revision 37
# speedup vs baseline: 7.4681x; 7.4681x over previous
"""Trainium2 SPMD kernel for H2OLlama attention (GQA + RoPE + causal softmax + o_proj).

Sharding (8 cores): core = b*4 + g  (b in {0,1} batch, g in {0..3} head group).
Each core handles one batch element, 8 q-heads and its 2 kv-heads; host sums the
4 o_proj partials per batch and transposes back.

Trace-driven changes vs the v2 baseline (773us NEFF, PE 92% busy):
- Diagonal narrowing: causal-diagonal score/AV matmuls (and their exp /
  mask / ptot ops) use rhs width 512-128a instead of 512 — removes the
  masked-out q columns from PE/ACT/DVE work (~20us PE, ~20us ACT).
- Startup: half-0 douts k0+k1+v0 run as a fused triple (v0 lagging 8
  hidden-chunks) so each arriving x chunk unlocks ~24 matmuls while the
  8MB x half streams in; idn/mask DMAs slotted behind the first chunks.
- o_proj weights are streamed per 256KB unit (6 in flight) instead of an
  8MB preload that could only DMA once proj SBUF freed (~205us) and
  stalled the PE at the phase transition. Half-1 x chunks ride the
  scalar queue so the sync queue stays clear for this stream.
- Softmax normalization chain of each attention block is deferred past
  the next block's score/mask ops (keeps the DVE FIFO short in front of
  the mask muls that gate AV matmuls); 10 o_proj units are emitted ahead
  of wave-B so the in-order PE stream has work while the last half-1 q
  ropes finish.
"""

import math
import sys

import numpy as np

sys.path.insert(0, "/opt/trn_rl_repo")

import ml_dtypes

import concourse.bass as bass
import concourse.tile as tile
from concourse import bacc, bass_isa, mybir
from concourse.bass_utils import run_bass_kernel_spmd

BF16 = mybir.dt.bfloat16
F32 = mybir.dt.float32

HIDDEN = 4096
N_HEADS = 32
N_KV_HEADS = 8
HEAD_DIM = 128
B, S = 2, 2048
ROPE_THETA = 10000.0

N_CORES = 8
HEADS_PER_CORE = N_HEADS // 4  # 8 q heads per core (4 head groups)
KV_PER_CORE = N_KV_HEADS // 4  # 2 kv heads per core
QDIM = HEADS_PER_CORE * HEAD_DIM  # 1024
KVDIM = KV_PER_CORE * HEAD_DIM  # 256
HC = HIDDEN // 128  # 32 hidden chunks
TT128 = S // 128  # 16 token tiles of 128
SCALE = 1.0 / math.sqrt(HEAD_DIM)

_BUILD_CACHE = {}


def _build_program():
    nc = bacc.Bacc("TRN2", target_bir_lowering=False, debug=False, num_devices=N_CORES)

    xt_d = nc.dram_tensor("xt", [128, HC, S], BF16, kind="ExternalInput")
    wq_d = nc.dram_tensor("wq", [HEADS_PER_CORE, 128, HC, 128], BF16, kind="ExternalInput")
    wk_d = nc.dram_tensor("wk", [KV_PER_CORE, 128, HC, 128], BF16, kind="ExternalInput")
    wv_d = nc.dram_tensor("wv", [KV_PER_CORE, 128, HC, 128], BF16, kind="ExternalInput")
    wo_d = nc.dram_tensor("wo", [128, HC, HEADS_PER_CORE, 128], BF16, kind="ExternalInput")
    cos_d = nc.dram_tensor("cosT", [128, S], F32, kind="ExternalInput")
    sin_d = nc.dram_tensor("sinT", [128, S], F32, kind="ExternalInput")
    rt_d = nc.dram_tensor("rt", [128, 128], BF16, kind="ExternalInput")
    idn_d = nc.dram_tensor("idn", [128, 128], BF16, kind="ExternalInput")
    mask_d = nc.dram_tensor("maskb", [128, 1024], BF16, kind="ExternalInput")
    out_d = nc.dram_tensor("out", [HIDDEN, S], BF16, kind="ExternalOutput")

    with tile.TileContext(nc) as tc:
        _kernel_body(nc, tc, xt_d, wq_d, wk_d, wv_d, wo_d, cos_d, sin_d, rt_d, idn_d, mask_d, out_d)

    nc.compile()
    return nc


def _kernel_body(nc, tc, xt_d, wq_d, wk_d, wv_d, wo_d, cos_d, sin_d, rt_d, idn_d, mask_d, out_d):
    EXP = mybir.ActivationFunctionType.Exp

    import contextlib

    with contextlib.ExitStack() as stack:
        ep = stack.enter_context
        qr_pool = ep(tc.tile_pool(name="qr", bufs=1))
        kr_pool = ep(tc.tile_pool(name="kr", bufs=1))
        v_pool = ep(tc.tile_pool(name="vv", bufs=1))
        a_pool = ep(tc.tile_pool(name="aa", bufs=1))
        const_pool = ep(tc.tile_pool(name="consts", bufs=1))
        # attention working pools (span wave A in proj scope and wave B after)
        p_pool = ep(tc.tile_pool(name="pp", bufs=5))
        ptot_pool = ep(tc.tile_pool(name="ptot", bufs=4))
        r_pool = ep(tc.tile_pool(name="rr", bufs=2))
        s_psum = ep(tc.tile_pool(name="sps", bufs=3, space="PSUM"))
        o_psum = ep(tc.tile_pool(name="ops", bufs=2, space="PSUM"))
        qr = [qr_pool.tile([128, S], BF16, tag=f"qr{h}", name=f"qr{h}") for h in range(HEADS_PER_CORE)]
        kr = [kr_pool.tile([128, S], BF16, tag=f"kr{k}", name=f"kr{k}") for k in range(KV_PER_CORE)]
        vt = v_pool.tile([128, TT128, KVDIM], BF16, tag="v", name="vt")
        at = [a_pool.tile([128, S], BF16, tag=f"a{h}", name=f"a{h}") for h in range(HEADS_PER_CORE)]

        rt_sb = const_pool.tile([128, 128], BF16, tag="rt", name="rt_sb")
        idn_sb = const_pool.tile([128, 128], BF16, tag="idn", name="idn_sb")
        mask_sb = const_pool.tile([128, 1024], BF16, tag="mask", name="mask_sb")
        ones_sb = const_pool.tile([128, 128], BF16, tag="ones", name="ones_sb")
        nc.vector.memset(ones_sb[:], 1.0)

        # ---------------- attention block emitter ----------------
        # The normalization chain (o_sb copy / row-sum / recip / at mul) of a
        # block is deferred until after the NEXT block's first scores+masks
        # are emitted: the AV matmuls gate on the mask muls, and this keeps
        # those at the front of the DVE FIFO instead of behind ~2us of f32.
        pending_fin = []

        def flush_fin():
            while pending_fin:
                pending_fin.pop(0)()

        def attn(h, qt):
            kvl = h // 4
            nj = 4 * qt + 4
            o_ps = o_psum.tile([128, 512], F32, tag="o", name="o_ps")
            ptot = ptot_pool.tile([128, 512], BF16, tag="ptot", name="ptot")
            pts = {}

            def emit_score(j):
                # diagonal blocks (a>=1): drop the leading a*128 q columns,
                # which are fully causal-masked anyway
                a = max(0, j - 4 * qt)
                w = 512 - a * 128
                s_ps = s_psum.tile([128, 512], F32, tag="s", name="s_ps")
                nc.tensor.matmul(
                    s_ps[:, 0:w],
                    lhsT=kr[kvl][:, j * 128 : (j + 1) * 128],
                    rhs=qr[h][:, qt * 512 + a * 128 : (qt + 1) * 512],
                    start=True,
                    stop=True,
                )
                p_t = p_pool.tile([128, 512], BF16, tag="p", name="p_t")
                nc.scalar.activation(p_t[:, 0:w], s_ps[:, 0:w], EXP, scale=SCALE)
                if j >= 4 * qt:  # diagonal block: multiplicative causal mask
                    nc.vector.tensor_mul(
                        p_t[:, 0:w], p_t[:, 0:w], mask_sb[:, 512 : 512 + w]
                    )
                pts[j] = (p_t, a, w)

            emit_score(0)
            emit_score(1)
            if nj > 2:
                emit_score(2)
            flush_fin()
            for j in range(nj):
                if j + 3 < nj:
                    emit_score(j + 3)
                p_t, a, w = pts.pop(j)
                nc.tensor.matmul(
                    o_ps[:, a * 128 : a * 128 + w],
                    lhsT=vt[:, j, kvl * 128 : (kvl + 1) * 128],
                    rhs=p_t[:, 0:w],
                    start=(j == 0),
                    stop=(j == nj - 1),
                )
                # ptot stays on DVE: GpSimd 2-input elementwise measured
                # ~1.08us per [128,512] op (3.5x DVE) and the serial chain
                # starved the PE (v4 experiment: 954us vs 736us)
                if j == 0:
                    nc.vector.tensor_copy(ptot[:], p_t[:])
                else:
                    nc.vector.tensor_add(
                        ptot[:, a * 128 : 512], ptot[:, a * 128 : 512], p_t[:, 0:w]
                    )
            # copy unnormalized output to SBUF immediately so the PSUM bank
            # recycles without waiting for the normalization chain
            o_sb = r_pool.tile([128, 512], F32, tag="osb", name="o_sb")
            nc.vector.tensor_copy(o_sb[:], o_ps[:])

            def fin():
                # fused row-sum + broadcast: ones.T @ ptot -> every row = r
                b_ps = s_psum.tile([128, 512], F32, tag="s", name="b_ps")
                nc.tensor.matmul(b_ps[:], lhsT=ones_sb[:], rhs=ptot[:], start=True, stop=True)
                ri = r_pool.tile([128, 512], F32, tag="ri", name="ri")
                nc.vector.reciprocal_approx_fast(ri[:], b_ps[:])
                nc.vector.tensor_mul(at[h][:, qt * 512 : (qt + 1) * 512], o_sb[:], ri[:])

            pending_fin.append(fin)

        # ================= projections (+ attention wave A) =================
        with (
            tc.tile_pool(name="xt", bufs=1) as x_pool,
            tc.tile_pool(name="wqk", bufs=3) as wqk_pool,
            tc.tile_pool(name="cs", bufs=1) as cs_pool,
            tc.tile_pool(name="raw", bufs=2) as raw_pool,
            tc.tile_pool(name="t1p", bufs=2) as t1_pool,
            tc.tile_pool(name="qkps", bufs=1, space="PSUM") as qk_psum,
            tc.tile_pool(name="rotps", bufs=1, space="PSUM") as rot_psum,
        ):
            for half in range(2):
                toff = half * 1024
                douts = (
                    [("k", i) for i in range(KV_PER_CORE)]
                    + [("v", i) for i in range(KV_PER_CORE)]
                    + [("q", i) for i in range(HEADS_PER_CORE)]
                )
                wsrc = {"k": wk_d, "v": wv_d, "q": wq_d}

                def w_dma(w_sb, kind, idx):
                    # split in two so the first LDWEIGHTS waits on half the bytes
                    src = wsrc[kind].ap()[idx]
                    nc.scalar.dma_start(w_sb[:, 0:16, :], src[:, 0:16, :])
                    nc.scalar.dma_start(w_sb[:, 16:HC, :], src[:, 16:HC, :])

                cos_sb = cs_pool.tile([128, 1024], F32, tag="cos", name="cos_sb")
                sin_sb = cs_pool.tile([128, 1024], F32, tag="sin", name="sin_sb")

                pre_w = []
                xs = []
                if half == 0:
                    # startup ordering (scalar queue || x chunks on sync):
                    # first-16-hc halves of w0/w1 race x0 so the k-pair can
                    # interleave from hc0; w2 (the lagged v0) next; cos/sin
                    # and idn/mask slot behind the first chunks
                    nc.scalar.dma_start(rt_sb[:], rt_d.ap())
                    w0 = wqk_pool.tile([128, HC, 128], BF16, tag="wqk", name="w_sb")
                    w1 = wqk_pool.tile([128, HC, 128], BF16, tag="wqk", name="w_sb")
                    w2 = wqk_pool.tile([128, HC, 128], BF16, tag="wqk", name="w_sb")
                    x0 = x_pool.tile([128, 4, 1024], BF16, tag="x0", name="x0")
                    nc.scalar.dma_start(w0[:, 0:16, :], wsrc[douts[0][0]].ap()[douts[0][1]][:, 0:16, :])
                    nc.scalar.dma_start(w1[:, 0:16, :], wsrc[douts[1][0]].ap()[douts[1][1]][:, 0:16, :])
                    nc.sync.dma_start(x0[:], xt_d.ap()[:, 0:4, toff : toff + 1024])
                    nc.scalar.dma_start(w0[:, 16:HC, :], wsrc[douts[0][0]].ap()[douts[0][1]][:, 16:HC, :])
                    nc.scalar.dma_start(w1[:, 16:HC, :], wsrc[douts[1][0]].ap()[douts[1][1]][:, 16:HC, :])
                    pre_w += [w0, w1, w2]
                    xs.append(x0)
                    x1 = x_pool.tile([128, 4, 1024], BF16, tag="x1", name="x1")
                    nc.sync.dma_start(x1[:], xt_d.ap()[:, 4:8, toff : toff + 1024])
                    xs.append(x1)
                    nc.scalar.dma_start(w2[:, 0:16, :], wsrc[douts[2][0]].ap()[douts[2][1]][:, 0:16, :])
                    nc.scalar.dma_start(cos_sb[:], cos_d.ap()[:, toff : toff + 1024])
                    nc.scalar.dma_start(sin_sb[:], sin_d.ap()[:, toff : toff + 1024])
                    x2 = x_pool.tile([128, 4, 1024], BF16, tag="x2", name="x2")
                    nc.sync.dma_start(x2[:], xt_d.ap()[:, 8:12, toff : toff + 1024])
                    xs.append(x2)
                    nc.scalar.dma_start(w2[:, 16:HC, :], wsrc[douts[2][0]].ap()[douts[2][1]][:, 16:HC, :])
                    nc.scalar.dma_start(idn_sb[:], idn_d.ap())
                    nc.scalar.dma_start(mask_sb[:], mask_d.ap())
                    for cg in range(3, 8):
                        xtile = x_pool.tile([128, 4, 1024], BF16, tag=f"x{cg}", name=f"x{cg}")
                        nc.sync.dma_start(
                            xtile[:], xt_d.ap()[:, cg * 4 : (cg + 1) * 4, toff : toff + 1024]
                        )
                        xs.append(xtile)
                else:
                    # half 1 runs entirely on the SCALAR queue (weights, cos/
                    # sin, and the x chunks): the SYNC queue stays empty for
                    # the streamed o_proj weight DMAs. Trigger ORDER is
                    # critical — the queue is FIFO and each trigger waits on
                    # its SBUF slot: only w0 (slot free ~150us) may precede
                    # the x triggers; w1/w2's and cos/sin's slots free with
                    # the LAST half-0 douts (~176us) and would block the
                    # whole 8MB x stream behind them.
                    w0 = wqk_pool.tile([128, HC, 128], BF16, tag="wqk", name="w_sb")
                    w_dma(w0, *douts[0])
                    pre_w.append(w0)
                    # reversed (chunk 7 first) to match the reversed hc sweep
                    xs = [None] * 8
                    for cg in range(7, -1, -1):
                        xtile = x_pool.tile([128, 4, 1024], BF16, tag=f"x{cg}", name=f"x{cg}")
                        nc.scalar.dma_start(
                            xtile[:], xt_d.ap()[:, cg * 4 : (cg + 1) * 4, toff : toff + 1024]
                        )
                        xs[cg] = xtile
                    for wi in range(1, 3):
                        w = wqk_pool.tile([128, HC, 128], BF16, tag="wqk", name="w_sb")
                        w_dma(w, *douts[wi])
                        pre_w.append(w)
                    nc.scalar.dma_start(cos_sb[:], cos_d.ap()[:, toff : toff + 1024])
                    nc.scalar.dma_start(sin_sb[:], sin_d.ap()[:, toff : toff + 1024])

                def xsl(hc, lo, sz):
                    return xs[hc // 4][:, hc % 4, lo : lo + sz]

                def rope_apply(dst, g0, src_ps):
                    raw = raw_pool.tile([128, 512], BF16, tag="raw", name="raw")
                    nc.vector.tensor_copy(raw[:], src_ps[:])
                    rot = rot_psum.tile([128, 512], F32, tag="rot", name="rot")
                    nc.tensor.matmul(rot[:], lhsT=rt_sb[:], rhs=raw[:], start=True, stop=True)
                    # t2 = rot * sin, in place in PSUM
                    nc.vector.tensor_mul(rot[:], rot[:], sin_sb[:, g0 : g0 + 512])
                    t1 = t1_pool.tile([128, 512], F32, tag="t1", name="t1")
                    nc.vector.tensor_mul(t1[:], raw[:], cos_sb[:, g0 : g0 + 512])
                    nc.vector.tensor_add(dst[:, toff + g0 : toff + g0 + 512], t1[:], rot[:])

                start_di = 0
                if half == 0:
                    # fused k0+k1+v0 start: interleave three douts' chains
                    # hc-major (v0 lagging 8 chunks so its weights have time
                    # to land) — each arriving x chunk unlocks ~24 matmuls
                    # during the x stream instead of 8, keeping the PE fed
                    # while the 8MB half streams in
                    w2 = pre_w[2]
                    pss0 = [
                        qk_psum.tile([128, 512], F32, tag=f"qk{tt}", name=f"qkps{tt}")
                        for tt in range(2)
                    ]
                    pss1 = [
                        s_psum.tile([128, 512], F32, tag="s", name=f"kpair{tt}")
                        for tt in range(2)
                    ]
                    pss2 = [
                        o_psum.tile([128, 512], F32, tag="o", name=f"vps{tt}")
                        for tt in range(2)
                    ]
                    LAG = 8
                    for hc in range(HC + LAG):
                        if hc < HC:
                            for tt in range(2):
                                nc.tensor.matmul(
                                    pss0[tt][:],
                                    lhsT=pre_w[0][:, hc, :],
                                    rhs=xsl(hc, tt * 512, 512),
                                    start=(hc == 0),
                                    stop=(hc == HC - 1),
                                )
                            for tt in range(2):
                                nc.tensor.matmul(
                                    pss1[tt][:],
                                    lhsT=pre_w[1][:, hc, :],
                                    rhs=xsl(hc, tt * 512, 512),
                                    start=(hc == 0),
                                    stop=(hc == HC - 1),
                                )
                        if hc >= LAG:
                            vc = hc - LAG
                            for tt in range(2):
                                nc.tensor.matmul(
                                    pss2[tt][:],
                                    lhsT=w2[:, vc, :],
                                    rhs=xsl(vc, tt * 512, 512),
                                    start=(vc == 0),
                                    stop=(vc == HC - 1),
                                )
                    for tt in range(2):
                        rope_apply(kr[0], tt * 512, pss0[tt])
                    for tt in range(2):
                        rope_apply(kr[1], tt * 512, pss1[tt])
                    # v0: transpose [d,tok] chunks into vt [tok,d]
                    for tt in range(2):
                        vraw = raw_pool.tile([128, 512], BF16, tag="raw", name="vraw")
                        nc.vector.tensor_copy(vraw[:], pss2[tt][:])
                        for c in range(4):
                            jb = tt * 4 + c
                            tp = s_psum.tile([128, 128], BF16, tag="s", name="tp")
                            nc.tensor.transpose(
                                tp[:], vraw[:, c * 128 : (c + 1) * 128], idn_sb[:]
                            )
                            nc.vector.tensor_copy(vt[:, jb, 0:128], tp[:])
                    start_di = 3

                wave_a = [(h, 0) for h in range(HEADS_PER_CORE)] + [
                    (h, 1) for h in range(HEADS_PER_CORE)
                ]
                # front-load ALL wave-A blocks before the first half-1 douts:
                # those douts crawl behind the 8MB x stream (~165-200us), and
                # the in-order PE stream can't pull later filler past them —
                # the scheduler's DMA model is too optimistic to do it for us
                wa_take = [6, 6, 4, 0, 0, 0, 0, 0, 0, 0, 0, 0]
                wa_off = [sum(wa_take[:i]) for i in range(len(wa_take))]
                for di in range(start_di, len(douts)):
                    kind, idx = douts[di]
                    if half == 1:
                        for hq in wave_a[wa_off[di] : wa_off[di] + wa_take[di]]:
                            attn(*hq)
                    if di < len(pre_w):
                        w_sb = pre_w[di]
                    else:
                        w_sb = wqk_pool.tile([128, HC, 128], BF16, tag="wqk", name="w_sb")
                        w_dma(w_sb, kind, idx)

                    pss = [
                        qk_psum.tile([128, 512], F32, tag=f"qk{tt}", name=f"qkps{tt}")
                        for tt in range(2)
                    ]
                    # half-0's last dout and all half-1 douts sweep hc in
                    # REVERSE: the last h0 dout then frees x-chunk 7 first,
                    # the (reversed) h1 x DMAs refill it first, and the
                    # (reversed) h1 douts consume it first — chunk free /
                    # refill / use pipeline instead of an 8MB barrier at the
                    # half boundary
                    rev = (half == 0 and di == len(douts) - 1) or half == 1
                    hcs = list(range(HC - 1, -1, -1)) if rev else list(range(HC))
                    for hi, hc in enumerate(hcs):
                        for tt in range(2):
                            nc.tensor.matmul(
                                pss[tt][:],
                                lhsT=w_sb[:, hc, :],
                                rhs=xsl(hc, tt * 512, 512),
                                start=(hi == 0),
                                stop=(hi == HC - 1),
                            )
                    if kind == "v":
                        # vT chunk [d=128, tok 512] -> transpose to vt [tok, d]
                        for tt in range(2):
                            vraw = raw_pool.tile([128, 512], BF16, tag="raw", name="vraw")
                            nc.vector.tensor_copy(vraw[:], pss[tt][:])
                            for c in range(4):
                                jb = half * 8 + tt * 4 + c
                                tp = s_psum.tile([128, 128], BF16, tag="s", name="tp")
                                nc.tensor.transpose(
                                    tp[:], vraw[:, c * 128 : (c + 1) * 128], idn_sb[:]
                                )
                                nc.vector.tensor_copy(
                                    vt[:, jb, idx * 128 : (idx + 1) * 128], tp[:]
                                )
                    else:
                        dst = qr[idx] if kind == "q" else kr[idx]
                        for tt in range(2):
                            rope_apply(dst, tt * 512, pss[tt])


        # ================= phase 2: wave B + o_proj =================
        with (
            tc.tile_pool(name="wo", bufs=6) as wo_pool,
            tc.tile_pool(name="oout", bufs=4) as out_pool,
            tc.tile_pool(name="outps", bufs=3, space="PSUM") as out_psum,
        ):
            # o_proj weights are STREAMED per unit (256KB each, 4 in flight)
            # instead of preloaded: the old 8MB preload could only DMA after
            # proj-phase SBUF freed (~205us), stalling the PE at the phase
            # transition. Re-fetching per tt costs 24MB extra HBM traffic —
            # well inside the bandwidth slack.
            def oproj_unit(tt, do):
                wo_t = wo_pool.tile(
                    [128, 1, HEADS_PER_CORE, 128], BF16, tag="wos", name="wo_t"
                )
                nc.sync.dma_start(wo_t[:], wo_d.ap()[:, do : do + 1, :, :])
                ps = out_psum.tile([128, 512], F32, tag="op", name="op")
                for a in range(HEADS_PER_CORE):
                    nc.tensor.matmul(
                        ps[:],
                        lhsT=wo_t[:, 0, a, :],
                        rhs=at[a][:, tt * 512 : (tt + 1) * 512],
                        start=(a == 0),
                        stop=(a == HEADS_PER_CORE - 1),
                    )
                ot = out_pool.tile([128, 512], BF16, tag="ot", name="ot")
                nc.vector.tensor_copy(ot[:], ps[:])
                nc.sync.dma_start(
                    out_d.ap()[do * 128 : (do + 1) * 128, tt * 512 : (tt + 1) * 512],
                    ot[:],
                )

            # interleave wave-B attention with o_proj units so the PE always
            # has filler work while exp/reduce chains run on ACT/GpSimd/DVE
            wave_b = [(h, 2) for h in range(HEADS_PER_CORE)] + [
                (h, 3) for h in range(HEADS_PER_CORE)
            ]
            units = [(tt, do) for tt in range(3) for do in range(HC)]  # tt=0,1,2
            ui = 0
            # pre-units BEFORE the first wave-B block: attn(0,2) waits on the
            # last half-1 q ropes (~215us) and the PE stream is in-order, so
            # anything emitted after it can't bypass the stall. These units
            # only need wave-A outputs (ready ~140us) + streamed wo weights.
            for _ in range(14):
                oproj_unit(*units[ui])
                ui += 1
            for gi, hq in enumerate(wave_b):
                attn(*hq)
                take = 3 if gi < 8 else 7
                for _ in range(take):
                    if ui < len(units):
                        oproj_unit(*units[ui])
                        ui += 1
            flush_fin()
            while ui < len(units):
                oproj_unit(*units[ui])
                ui += 1
            for do in range(HC):
                oproj_unit(3, do)


# ======================= host-side sharding =======================


def _rope_tables(position_ids_b):
    pos = position_ids_b.astype(np.float32)  # [S]
    inv_freq = 1.0 / (ROPE_THETA ** (np.arange(0, HEAD_DIM, 2, dtype=np.float32) / HEAD_DIM))
    freqs = pos[:, None] * inv_freq[None, :]  # [S, 64]
    emb = np.concatenate([freqs, freqs], axis=1)  # [S, 128]
    cosT = np.ascontiguousarray(np.cos(emb).T.astype(np.float32))  # [128, S]
    sinT = np.ascontiguousarray(np.sin(emb).T.astype(np.float32))
    return cosT, sinT


def _shared_consts():
    rt = np.zeros((128, 128), dtype=ml_dtypes.bfloat16)
    idx = np.arange(64)
    rt[idx, idx + 64] = 1.0  # RT[j, j+64] = +1  (j < 64)
    rt[idx + 64, idx] = -1.0  # RT[j+64, j] = -1
    idn = np.eye(128, dtype=ml_dtypes.bfloat16)
    maskb = np.zeros((128, 1024), dtype=ml_dtypes.bfloat16)
    k = np.arange(128)[:, None]
    c = np.arange(1024)[None, :]
    maskb[:] = (c >= k + 512).astype(ml_dtypes.bfloat16)
    return rt, idn, maskb


def kernel(hidden_states, position_ids, Wq, Wk, Wv, Wo):
    bf16 = ml_dtypes.bfloat16
    if "nc" not in _BUILD_CACHE:
        _BUILD_CACHE["nc"] = _build_program()
    nc = _BUILD_CACHE["nc"]

    rt, idn, maskb = _shared_consts()
    Wq16, Wk16, Wv16, Wo16 = (w.astype(bf16) for w in (Wq, Wk, Wv, Wo))

    xts, coss, sins = [], [], []
    for b in range(B):
        xb = np.asarray(hidden_states[b], dtype=np.float32).T.astype(bf16)  # [4096, S]
        xt = np.ascontiguousarray(xb.reshape(HC, 128, S).transpose(1, 0, 2))  # [128, 32, S]
        xts.append(xt)
        cosT, sinT = _rope_tables(np.asarray(position_ids[b]))
        coss.append(cosT)
        sins.append(sinT)

    in_maps = []
    for core in range(N_CORES):
        b, g = core // 4, core % 4
        wq = np.ascontiguousarray(
            Wq16[:, g * QDIM : (g + 1) * QDIM].reshape(HC, 128, HEADS_PER_CORE, 128).transpose(2, 1, 0, 3)
        )
        wk = np.ascontiguousarray(
            Wk16[:, g * KVDIM : (g + 1) * KVDIM].reshape(HC, 128, KV_PER_CORE, 128).transpose(2, 1, 0, 3)
        )
        wv = np.ascontiguousarray(
            Wv16[:, g * KVDIM : (g + 1) * KVDIM].reshape(HC, 128, KV_PER_CORE, 128).transpose(2, 1, 0, 3)
        )
        wo = np.ascontiguousarray(
            Wo16[g * QDIM : (g + 1) * QDIM, :].reshape(HEADS_PER_CORE, 128, HC, 128).transpose(1, 2, 0, 3)
        )
        in_maps.append(
            {
                "xt": xts[b],
                "wq": wq,
                "wk": wk,
                "wv": wv,
                "wo": wo,
                "cosT": coss[b],
                "sinT": sins[b],
                "rt": rt,
                "idn": idn,
                "maskb": maskb,
            }
        )

    res = run_bass_kernel_spmd(nc, in_maps, list(range(N_CORES))).results

    out = np.empty((B, S, HIDDEN), dtype=np.float32)
    for b in range(B):
        acc = res[4 * b]["out"].astype(np.float32)
        for g in range(1, 4):
            acc = acc + res[4 * b + g]["out"]
        out[b] = acc.T
    return out
